# revision 1
# baseline (speedup 1.0000x reference)
"""EngagementPredictor TRN2 kernel: 3-branch MHA + masked mean-pool + MLP.

Sharding: pure data-parallel — B=8 batch elements, one per NeuronCore;
weights replicated; no collectives. Each core computes its [2]-logit row.

Per-core dataflow (S=1024, H=1024, fp32 storage / float32r matmuls):
  xT [H,S] resident in SBUF. For each MHA branch (beh 8h / tmp 4h / pat 4h):
    QT,KT [H,S] = W^T-stationary projections (Q gets its bias fused into the
    PSUM->SBUF evacuation; K bias dropped — softmax is invariant to per-q
    constants; V bias folded into the pooled vector).
    V [S,H] natural-layout projection.
    Attention in transposed layout: scoresT[k,q] per (head, q-chunk, k-tile),
    Exp fused with 1/sqrt(d) scale and the key mask as a per-partition bias
    (-30000 on masked keys -> exact zero probs). ctxT[d,q] = V^T @ expT needs
    no transposes. Softmax denominator via ones-column matmuls; the pooling
    weights mask[q]/(mask_sum*denom[q]) are broadcast across partitions with a
    K=1 matmul; masked mean-pool runs on DVE directly off PSUM.
    o-projection applied AFTER pooling (1xH instead of SxH).
    fus1 partial products accumulate per-branch so fus1_w streams during
    compute instead of serializing at the tail.
  Tail: relu MLP (fus1/fus2/cls) at M=1.
"""
import numpy as np

import concourse.bass as bass
import concourse.tile as tile
from concourse import mybir
from concourse.bass_utils import run_bass_kernel_spmd

F32 = mybir.dt.float32
F32R = mybir.dt.float32r
AF = mybir.ActivationFunctionType
ALU = mybir.AluOpType

P = 128
S = 1024
H = 1024
NT = H // P          # 8 tiles of 128 along H or S
QC = 512             # chunk width for projections / o-proj / MLP
NQC = S // QC        # 2
QCA = 256            # attention q-chunk width (SBUF-budget bound)
NQCA = S // QCA      # 4
NCORES = 8
MHAS = [("beh", 8), ("tmp", 4), ("pat", 4)]

_CACHE = {}


def _build_nc():
    nc = bass.Bass()
    dram = {}

    def dp(name, shape):
        dram[name] = nc.declare_dram_parameter(name, list(shape), F32,
                                               isOutput=False)

    dp("xT", (H, S))
    dp("maskb", (P, NT))       # -30000/0 per key position, partition-inner
    dp("poolw", (1, S))        # mask / mask_sum
    dp("ones", (P,))
    for m, _ in MHAS:
        for wn in ("qw", "kw", "vw", "ow"):
            dp(f"{m}_{wn}", (H, H))
        dp(f"{m}_qb", (P, NT))
        dp(f"{m}_vb", (P, NT))
        dp(f"{m}_ob", (P, NT))
    dp("fus1_w", (3 * H, H))
    dp("fus1_b", (P, NT))
    dp("fus2_w", (H, H // 2))
    dp("fus2_b", (P, 4))
    dp("cls_w", (H // 2, 2))
    dp("cls_b", (1, 2))
    out = nc.declare_dram_parameter("out", [1, 2], F32, isOutput=True)

    def r3(ap):  # [K, N] dram -> [P, K//P, N] partition-inner
        return ap[:].rearrange("(t p) n -> p t n", p=P)

    with tile.TileContext(nc) as tc, \
         nc.allow_low_precision(
             reason="float32r tiles: fp32 bits with mantissa rounding on "
                    "write; DVE reduces accumulate at fp32 internally"):
        with tc.tile_pool(name="big", bufs=1) as big, \
             tc.tile_pool(name="wstr", bufs=3) as wstr, \
             tc.tile_pool(name="expp", bufs=2) as expp, \
             tc.tile_pool(name="small", bufs=1) as small, \
             tc.tile_pool(name="work", bufs=2) as work:

            # ---- resident inputs ----
            xT = big.tile([P, NT, S], F32R, tag="xT")
            nc.sync.dma_start(xT[:], r3(dram["xT"]).bitcast(F32R))
            QT = big.tile([P, NT, S], F32R, tag="QT")
            KT = big.tile([P, NT, S], F32R, tag="KT")
            V = big.tile([P, NT, H], F32R, tag="V")

            mb = small.tile([P, NT], F32, tag="mb")
            nc.sync.dma_start(mb[:], dram["maskb"][:])
            pw = small.tile([1, S], F32, tag="pw")
            nc.sync.dma_start(pw[:], dram["poolw"][:])
            ones_col = small.tile([P, 1], F32R, tag="ones_col")
            nc.sync.dma_start(ones_col[:], dram["ones"][:, None].bitcast(F32R))
            ones_row = small.tile([1, P], F32R, tag="ones_row")
            nc.sync.dma_start(ones_row[:], dram["ones"][None, :].bitcast(F32R))

            # h1 pre-activation accumulated in column layout [P, NT]
            h1acc = small.tile([P, NT], F32, tag="h1acc")
            nc.vector.memset(h1acc[:], 0.0)

            for mi, (m, nh) in enumerate(MHAS):
                d = H // nh
                ndt = d // P
                inv_sqrt_d = 1.0 / float(np.sqrt(d))

                qb = small.tile([P, NT], F32, tag="qb")
                nc.sync.dma_start(qb[:], dram[f"{m}_qb"][:])
                vb = small.tile([P, NT], F32, tag="vb")
                nc.sync.dma_start(vb[:], dram[f"{m}_vb"][:])
                ob = small.tile([P, NT], F32, tag="ob")
                nc.sync.dma_start(ob[:], dram[f"{m}_ob"][:])

                # ---------- projections ----------
                with tc.tile_pool(name=f"pj{mi}", bufs=8, space="PSUM") as pj:
                    # Q and K: out[ho, s] ; lhsT = w[ki, ho-slice] (stationary)
                    for wn, dst, with_bias in ((f"{m}_qw", QT, True),
                                               (f"{m}_kw", KT, False)):
                        wr = r3(dram[wn]).bitcast(F32R)
                        for hog in range(2):
                            pst = [pj.tile([P, QC], F32, tag="pj",
                                           name=f"pj{mi}_{wn}_{hog}_{i}")
                                   for i in range(8)]
                            for ki in range(NT):
                                wt = wstr.tile([P, H], F32R, tag="w")
                                nc.sync.dma_start(wt[:], wr[:, ki])
                                for ho4 in range(4):
                                    ho = hog * 4 + ho4
                                    hsl = slice(ho * P, (ho + 1) * P)
                                    for qc in range(NQC):
                                        qsl = slice(qc * QC, (qc + 1) * QC)
                                        nc.tensor.matmul(
                                            pst[ho4 * 2 + qc][:],
                                            lhsT=wt[:, hsl],
                                            rhs=xT[:, ki, qsl],
                                            start=(ki == 0),
                                            stop=(ki == NT - 1))
                            for ho4 in range(4):
                                ho = hog * 4 + ho4
                                for qc in range(NQC):
                                    qsl = slice(qc * QC, (qc + 1) * QC)
                                    pt = pst[ho4 * 2 + qc]
                                    if with_bias:
                                        nc.scalar.activation(
                                            dst[:, ho, qsl], pt[:], AF.Identity,
                                            bias=qb[:, ho:ho + 1], scale=1.0)
                                    else:
                                        nc.vector.tensor_copy(
                                            dst[:, ho, qsl], pt[:])
                    # V: out[s, h] ; lhsT = xT[ki, s-slice] (stationary)
                    vr = r3(dram[f"{m}_vw"]).bitcast(F32R)
                    for sg in range(2):
                        pst = [pj.tile([P, QC], F32, tag="pj",
                                       name=f"pjv{mi}_{sg}_{i}")
                               for i in range(8)]
                        for ki in range(NT):
                            wt = wstr.tile([P, H], F32R, tag="w")
                            nc.sync.dma_start(wt[:], vr[:, ki])
                            for s4 in range(4):
                                st = sg * 4 + s4
                                ssl = slice(st * P, (st + 1) * P)
                                for hc in range(NQC):
                                    hsl = slice(hc * QC, (hc + 1) * QC)
                                    nc.tensor.matmul(
                                        pst[s4 * 2 + hc][:],
                                        lhsT=xT[:, ki, ssl],
                                        rhs=wt[:, hsl],
                                        start=(ki == 0),
                                        stop=(ki == NT - 1))
                        for s4 in range(4):
                            st = sg * 4 + s4
                            for hc in range(NQC):
                                hsl = slice(hc * QC, (hc + 1) * QC)
                                nc.vector.tensor_copy(
                                    V[:, st, hsl], pst[s4 * 2 + hc][:])

                # ---------- attention + pool + o-proj + fus1 partial ----------
                with tc.tile_pool(name=f"sc{mi}", bufs=2, space="PSUM") as psc, \
                     tc.tile_pool(name=f"cx{mi}", bufs=1, space="PSUM") as pcx, \
                     tc.tile_pool(name=f"dn{mi}", bufs=1, space="PSUM") as pdn, \
                     tc.tile_pool(name=f"wb{mi}", bufs=1, space="PSUM") as pwb, \
                     tc.tile_pool(name=f"po{mi}", bufs=2, space="PSUM") as ppo:
                    pooled = small.tile([P, NT], F32R, tag="pooled")
                    for qc in range(NQCA):
                        qsl = slice(qc * QCA, (qc + 1) * QCA)
                        for h in range(nh):
                            expt = expp.tile([P, NT, QCA], F32R, tag="expt")
                            for kt in range(NT):
                                ksl = slice(kt * P, (kt + 1) * P)
                                ssc = psc.tile([P, QCA], F32, tag="sc")
                                for dt in range(ndt):
                                    nc.tensor.matmul(
                                        ssc[:],
                                        lhsT=KT[:, h * ndt + dt, ksl],
                                        rhs=QT[:, h * ndt + dt, qsl],
                                        start=(dt == 0),
                                        stop=(dt == ndt - 1))
                                nc.scalar.activation(
                                    expt[:, kt], ssc[:], AF.Exp,
                                    bias=mb[:, kt:kt + 1], scale=inv_sqrt_d)
                            sdn = pdn.tile([1, QCA], F32, tag="dn")
                            for kt in range(NT):
                                nc.tensor.matmul(
                                    sdn[:], lhsT=ones_col[:], rhs=expt[:, kt],
                                    start=(kt == 0), stop=(kt == NT - 1))
                            recip = work.tile([1, QCA], F32, tag="recip")
                            nc.vector.reciprocal(recip[:], sdn[:])
                            w = work.tile([1, QCA], F32R, tag="w")
                            nc.vector.tensor_mul(out=w[:], in0=recip[:],
                                                 in1=pw[:, qsl])
                            swb = pwb.tile([P, QCA], F32, tag="wb")
                            nc.tensor.matmul(swb[:], lhsT=ones_row[:],
                                             rhs=w[:], start=True, stop=True)
                            wb_sb = work.tile([P, QCA], F32, tag="wb_sb")
                            nc.vector.tensor_copy(wb_sb[:], swb[:])
                            for dt in range(ndt):
                                gdt = h * ndt + dt
                                dsl = slice(gdt * P, (gdt + 1) * P)
                                sctx = pcx.tile([P, QCA], F32, tag="cx")
                                for kt in range(NT):
                                    nc.tensor.matmul(
                                        sctx[:], lhsT=V[:, kt, dsl],
                                        rhs=expt[:, kt],
                                        start=(kt == 0), stop=(kt == NT - 1))
                                prod = work.tile([P, QCA], F32, tag="prod")
                                nc.vector.tensor_mul(out=prod[:], in0=sctx[:],
                                                     in1=wb_sb[:])
                                if qc == 0:
                                    nc.vector.tensor_reduce(
                                        pooled[:, gdt:gdt + 1], prod[:],
                                        axis=mybir.AxisListType.X, op=ALU.add)
                                else:
                                    pp = work.tile([P, 1], F32, tag="pp")
                                    nc.vector.tensor_reduce(
                                        pp[:], prod[:],
                                        axis=mybir.AxisListType.X, op=ALU.add)
                                    nc.vector.tensor_add(
                                        out=pooled[:, gdt:gdt + 1],
                                        in0=pooled[:, gdt:gdt + 1], in1=pp[:])
                    # + V bias (exact: pooling weights sum to 1)
                    nc.vector.tensor_add(out=pooled[:], in0=pooled[:],
                                         in1=vb[:])
                    # o-projection, column layout: fTm[p,t] = (pooled@ow)[t*P+p]
                    # lhsT = ow k-tile column block (stationary), rhs = pooled
                    # column (N=1). ob fused into the PSUM evacuation.
                    owr = r3(dram[f"{m}_ow"]).bitcast(F32R)
                    fTm = small.tile([P, NT], F32R, tag="fTm")
                    for tg in range(4):
                        pos = [ppo.tile([P, 1], F32, tag="po",
                                        name=f"po{mi}_{tg}_{i}")
                               for i in range(2)]
                        for ki in range(NT):
                            owt = wstr.tile([P, H], F32R, tag="w")
                            nc.sync.dma_start(owt[:], owr[:, ki])
                            for t2 in range(2):
                                t = tg * 2 + t2
                                nc.tensor.matmul(
                                    pos[t2][:],
                                    lhsT=owt[:, t * P:(t + 1) * P]
                                    .bitcast(F32),
                                    rhs=pooled[:, ki:ki + 1].bitcast(F32),
                                    start=(ki == 0), stop=(ki == NT - 1))
                        for t2 in range(2):
                            t = tg * 2 + t2
                            nc.scalar.activation(
                                fTm[:, t:t + 1], pos[t2][:], AF.Identity,
                                bias=ob[:, t:t + 1], scale=1.0)
                    # fus1 partial: h1acc += fused[m-part] @ fus1_w[m-rows]
                    w1r = r3(dram["fus1_w"]).bitcast(F32R)
                    for tg in range(4):
                        ph1 = [ppo.tile([P, 1], F32, tag="po",
                                        name=f"ph1_{mi}_{tg}_{i}")
                               for i in range(2)]
                        for ki in range(NT):
                            w1t = wstr.tile([P, H], F32R, tag="w")
                            nc.sync.dma_start(w1t[:], w1r[:, mi * NT + ki])
                            for t2 in range(2):
                                t = tg * 2 + t2
                                nc.tensor.matmul(
                                    ph1[t2][:],
                                    lhsT=w1t[:, t * P:(t + 1) * P]
                                    .bitcast(F32),
                                    rhs=fTm[:, ki:ki + 1].bitcast(F32),
                                    start=(ki == 0), stop=(ki == NT - 1))
                        for t2 in range(2):
                            t = tg * 2 + t2
                            nc.vector.tensor_add(
                                out=h1acc[:, t:t + 1], in0=ph1[t2][:],
                                in1=h1acc[:, t:t + 1])

            # ---------- MLP tail ----------
            with tc.tile_pool(name="tail", bufs=2, space="PSUM") as ptl:
                b1 = small.tile([P, NT], F32, tag="b1")
                nc.sync.dma_start(b1[:], dram["fus1_b"][:])
                h1pre = small.tile([P, NT], F32, tag="h1pre")
                nc.vector.tensor_add(out=h1pre[:], in0=h1acc[:], in1=b1[:])
                h1T = small.tile([P, NT], F32R, tag="h1T")
                nc.scalar.activation(h1T[:], h1pre[:], AF.Relu)

                w2r = r3(dram["fus2_w"]).bitcast(F32R)  # [P, 8, 512]
                b2 = small.tile([P, 4], F32, tag="b2")
                nc.sync.dma_start(b2[:], dram["fus2_b"][:])
                h2T = small.tile([P, 4], F32R, tag="h2T")
                for tg in range(2):
                    ph2 = [ptl.tile([P, 1], F32, tag="t2",
                                    name=f"ph2_{tg}_{i}") for i in range(2)]
                    for ki in range(NT):
                        w2t = wstr.tile([P, QC], F32R, tag="w2")
                        nc.sync.dma_start(w2t[:], w2r[:, ki])
                        for t2 in range(2):
                            t = tg * 2 + t2
                            nc.tensor.matmul(
                                ph2[t2][:],
                                lhsT=w2t[:, t * P:(t + 1) * P].bitcast(F32),
                                rhs=h1T[:, ki:ki + 1].bitcast(F32),
                                start=(ki == 0), stop=(ki == NT - 1))
                    for t2 in range(2):
                        t = tg * 2 + t2
                        nc.scalar.activation(h2T[:, t:t + 1], ph2[t2][:],
                                             AF.Relu, bias=b2[:, t:t + 1],
                                             scale=1.0)

                cwr = r3(dram["cls_w"]).bitcast(F32R)  # [P, 4, 2]
                cwt = small.tile([P, 4, 2], F32R, tag="cwt")
                nc.sync.dma_start(cwt[:], cwr)
                plg = ptl.tile([1, 2], F32, tag="lg")
                for ki in range(4):
                    nc.tensor.matmul(plg[:],
                                     lhsT=h2T[:, ki:ki + 1].bitcast(F32),
                                     rhs=cwt[:, ki].bitcast(F32),
                                     start=(ki == 0), stop=(ki == 3))
                cb = small.tile([1, 2], F32, tag="cb")
                nc.sync.dma_start(cb[:], dram["cls_b"][:])
                lg = small.tile([1, 2], F32, tag="lgsb")
                nc.vector.tensor_add(out=lg[:], in0=plg[:], in1=cb[:])
                nc.sync.dma_start(out[:], lg[:])

    _split_multi_waits(nc)
    return nc


def _split_multi_waits(nc, max_on_inst=1, max_on_evsem=2):
    """This walrus build caps sync waits per instruction at 1 (2 for
    EventSemaphore); Tile attaches one wait per dependent proc. Spill excess
    waits onto pure-wait EventSemaphores inserted before, on the same engine —
    the engine blocks on each condition in sequence, so semantics match."""
    for f in nc.m.functions:
        for bb in f.blocks:
            insts = list(bb.instructions)
            new = []
            changed = False
            for ins in insts:
                si = ins.sync_info
                if si is not None:
                    waits = list(si.on_wait)
                    cap = (max_on_evsem
                           if isinstance(ins, mybir.InstEventSemaphore)
                           else max_on_inst)
                    if len(waits) > cap:
                        spill = waits[:-cap]
                        keep = waits[-cap:]
                        k = 0
                        while spill:
                            chunk = spill[:max_on_evsem]
                            spill = spill[max_on_evsem:]
                            new.append(mybir.InstEventSemaphore(
                                name=f"{ins.name}-wspill{k}",
                                engine=ins.engine, ins=[], outs=[],
                                sync_info=mybir.SyncInfo(on_wait=chunk,
                                                         on_update=[])))
                            k += 1
                        ins.sync_info = mybir.SyncInfo(
                            on_wait=keep, on_update=list(si.on_update))
                        changed = True
                new.append(ins)
            if changed:
                bb.instructions = new


def _get_nc():
    if "nc" not in _CACHE:
        _CACHE["nc"] = _build_nc()
    return _CACHE["nc"]


def _prep_in_maps(inputs):
    f32 = np.float32
    mask = inputs["attention_mask"].astype(f32)          # [B, S]
    denom = mask.sum(axis=1, keepdims=True)              # [B, 1]
    poolw = (mask / denom).astype(f32)                   # [B, S]
    maskb = np.where(mask > 0, 0.0, -30000.0).astype(f32)  # [B, S]

    shared = {"ones": np.ones(P, f32)}
    for m, _ in MHAS:
        for wn in ("qw", "kw", "vw", "ow"):
            shared[f"{m}_{wn}"] = np.ascontiguousarray(
                inputs[f"{m}_{wn}"], dtype=f32)
        shared[f"{m}_qb"] = np.ascontiguousarray(
            inputs[f"{m}_qb"].astype(f32).reshape(NT, P).T)
        shared[f"{m}_vb"] = np.ascontiguousarray(
            inputs[f"{m}_vb"].astype(f32).reshape(NT, P).T)
        shared[f"{m}_ob"] = np.ascontiguousarray(
            inputs[f"{m}_ob"].astype(f32).reshape(NT, P).T)
    shared["fus1_w"] = np.ascontiguousarray(inputs["fus1_w"], dtype=f32)
    shared["fus1_b"] = np.ascontiguousarray(
        inputs["fus1_b"].astype(f32).reshape(NT, P).T)
    shared["fus2_w"] = np.ascontiguousarray(inputs["fus2_w"], dtype=f32)
    shared["fus2_b"] = np.ascontiguousarray(
        inputs["fus2_b"].astype(f32).reshape(4, P).T)
    shared["cls_w"] = np.ascontiguousarray(inputs["cls_w"], dtype=f32)
    shared["cls_b"] = inputs["cls_b"].astype(f32).reshape(1, 2)

    in_maps = []
    for c in range(NCORES):
        im = dict(shared)
        im["xT"] = np.ascontiguousarray(
            inputs["hidden_states"][c].astype(f32).T)
        im["maskb"] = np.ascontiguousarray(maskb[c].reshape(NT, P).T)
        im["poolw"] = poolw[c].reshape(1, S)
        in_maps.append(im)
    return in_maps


def kernel(**inputs) -> np.ndarray:
    nc = _get_nc()
    in_maps = _prep_in_maps(inputs)
    res = run_bass_kernel_spmd(nc, in_maps, core_ids=list(range(NCORES)))
    return np.concatenate(
        [res.results[c]["out"] for c in range(NCORES)], axis=0
    ).astype(np.float32)

